# revision 7
# baseline (speedup 1.0000x reference)
"""MLA prefill kernel for Trainium2, tensor-parallel over heads on 8 NeuronCores.

Fully fused single-pass pipeline:
- Projections write q/k/v directly into persistent per-batch SBUF tiles in
  exactly the layout attention consumes (no DRAM roundtrip for
  intermediates). Attention chunk (b, qb) runs right after projection
  row-block (b, rb=qb) - causality means its whole K/V prefix is ready - so
  the tensor engine alternates between projection and attention chains with
  no phase boundaries (TimelineSim: ~89% PE busy).
- bf16 datapath (same PE rate as fp32r, half the DMA/SBUF), fp32 PSUM
  accumulation, fp32 softmax denominator (two interleaved DVE accumulators +
  one ones-matmul per head/q-block for the partition reduce).
- Both heads advance together per k-tile: their K=64 rope-score matmuls use
  disjoint PE row groups (partitions 0-63 / 64-127) and issue back-to-back
  for PE-array concurrency. The k_pe weight columns carry a built-in
  duplicate, so the projection emits k_pe into both partition halves at the
  same matmul cost and no SBUF->SBUF dup DMA sits on the attention path.
- Diagonal k-tiles slice the fully-masked dead columns out of the exp, the
  PV matmul and the rowsum accumulation (PSUM has_written keeps untouched
  columns' partials); causal mask via DVE multiply with bf16 0/1 tiles.
- rmsnorm rsqrt computed as exp(-0.5*ln(y)): ln and exp share the single
  natural_log_exp ACT table set, so the interleaved attention exp never
  thrashes the activation tables.
- PSUM-evacuation copies are balanced across the scalar and vector engines
  (k_nope to DVE, output-projection tiles to ACT) to keep both off the
  attention chains' critical path.
- Chunked startup loads overlap the first projection chain with the initial
  DMA; x row-blocks are prefetched one block ahead.

Contract: kernel(**inputs) takes FULL unsharded inputs, returns FULL [B,S,D]
float32. 16 heads -> 2 per core; each core computes a partial output
projection [D, B*S] (transposed, bf16); host sums the 8 partials.
"""
import sys
sys.path.insert(0, '/opt/trn_rl_repo')
import numpy as np

B, S, D = 4, 2048, 2048
H = 16
NOPE, ROPE, VD = 128, 64, 128
QK = NOPE + ROPE
KVR = 512
THETA = 10000.0
EPS = 1e-6
SCALE = QK ** -0.5
NCORES = 8
HPC = H // NCORES          # heads per core = 2
R = B * S                  # 8192 flattened rows
RB = 512                   # row block == attention q block
NRB = R // RB              # 16
QB = 512
NQB = S // QB              # 4 per batch

_cache = {}


def _build_nc():
    import concourse.bass as bass  # noqa: F401
    import concourse.mybir as mybir
    import concourse.tile as tile
    from concourse import bacc
    from contextlib import ExitStack

    F32 = mybir.dt.float32
    F32R = mybir.dt.float32r
    BF16 = mybir.dt.bfloat16
    EXP = mybir.ActivationFunctionType.Exp
    LN = mybir.ActivationFunctionType.Ln
    MULT = mybir.AluOpType.mult
    ADD = mybir.AluOpType.add
    GE = mybir.AluOpType.is_ge

    nc = bacc.Bacc("TRN2", target_bir_lowering=False, debug=False)

    xT = nc.dram_tensor("xT", [D, R], BF16, kind="ExternalInput")
    wqT = nc.dram_tensor("wqT", [D, 3 * 128], BF16, kind="ExternalInput")
    wkvT = nc.dram_tensor("wkvT", [D, KVR + 2 * ROPE], BF16, kind="ExternalInput")
    wbkT = nc.dram_tensor("wbkT", [KVR, 2 * NOPE], BF16, kind="ExternalInput")
    wbvT = nc.dram_tensor("wbvT", [KVR, 2 * VD], BF16, kind="ExternalInput")
    woT = nc.dram_tensor("woT", [2 * VD, D], BF16, kind="ExternalInput")
    ropeC = nc.dram_tensor("ropeC", [128, S], BF16, kind="ExternalInput")
    ropeS = nc.dram_tensor("ropeS", [128, S], BF16, kind="ExternalInput")
    perm = nc.dram_tensor("perm", [128, 128], BF16, kind="ExternalInput")
    onesw = nc.dram_tensor("onesw", [128, 128], F32R, kind="ExternalInput")
    masks = nc.dram_tensor("masks", [128, 4 * QB], BF16, kind="ExternalInput")
    pout = nc.dram_tensor("pout", [D, R], BF16, kind="ExternalOutput")

    xT_r = xT.ap().rearrange("(co ci) r -> ci co r", ci=128)      # [128,16,R]
    wqT_r = wqT.ap().rearrange("(co ci) f -> ci co f", ci=128)    # [128,16,384]
    wkvT_r = wkvT.ap().rearrange("(co ci) f -> ci co f", ci=128)  # [128,16,576]
    wbkT_r = wbkT.ap().rearrange("(co ci) f -> ci co f", ci=128)  # [128,4,256]
    wbvT_r = wbvT.ap().rearrange("(co ci) f -> ci co f", ci=128)
    woT_r = woT.ap().rearrange("(hc hi) d -> hi hc d", hi=128)    # [128,2,D]

    with tile.TileContext(nc) as tc:
        with ExitStack() as stk:
            gpool = stk.enter_context(tc.tile_pool(name="gconst", bufs=1))
            wp = stk.enter_context(tc.tile_pool(name="p1w", bufs=1))
            sp = stk.enter_context(tc.tile_pool(name="p1sb", bufs=2))
            xp = stk.enter_context(tc.tile_pool(name="p1x", bufs=2))
            kvp = stk.enter_context(tc.tile_pool(name="pkv", bufs=2))
            ep3 = stk.enter_context(tc.tile_pool(name="p3ex", bufs=4))
            sp3 = stk.enter_context(tc.tile_pool(name="p3sb", bufs=2))
            op3 = stk.enter_context(tc.tile_pool(name="p3o", bufs=2))
            sp4 = stk.enter_context(tc.tile_pool(name="p4sb", bufs=4))
            psA = stk.enter_context(tc.tile_pool(name="p1ps", bufs=2, space="PSUM"))
            psX = stk.enter_context(tc.tile_pool(name="p1aux", bufs=1, space="PSUM"))
            pss = stk.enter_context(tc.tile_pool(name="p3ps", bufs=3, space="PSUM"))
            pso = stk.enter_context(tc.tile_pool(name="p3ps2", bufs=2, space="PSUM"))

            perm_sb = gpool.tile([128, 128], BF16)
            ones_sb = gpool.tile([128, 128], F32R)
            eps_sb = gpool.tile([128, 1], F32)
            wo_sb = gpool.tile([128, 2, D], BF16)
            mask_sb = gpool.tile([128, 4 * QB], BF16)

            wq_sb = [wp.tile([128, 384], BF16, tag=f"wq{dc}", name=f"wq{dc}") for dc in range(16)]
            wkv_sb = [wp.tile([128, 640], BF16, tag=f"wkv{dc}", name=f"wkv{dc}") for dc in range(16)]
            wbk_sb = wp.tile([128, 4, 256], BF16)
            wbv_sb = wp.tile([128, 4, 256], BF16)
            ropeC_sb = wp.tile([128, S], BF16)
            ropeS_sb = wp.tile([128, S], BF16)

            # first q-chain consumes wq[dc]+xt[dc] in dc order: issue those
            # first (HWDGE drains in FIFO order), bulk weights after
            xt0 = [xp.tile([128, RB], BF16, tag=f"xt{dc}", name=f"xt0_{dc}")
                   for dc in range(16)]
            nc.vector.memset(eps_sb, EPS)
            fill0 = nc.gpsimd.to_reg(0.0)
            for dc in range(16):
                nc.sync.dma_start(wq_sb[dc], wqT_r[:, dc])
                nc.sync.dma_start(xt0[dc], xT_r[:, dc, 0:RB])
            for dc in range(16):
                nc.sync.dma_start(wkv_sb[dc], wkvT_r[:, dc])
            nc.sync.dma_start(perm_sb, perm.ap())
            nc.sync.dma_start(ones_sb, onesw.ap())
            nc.sync.dma_start(wbk_sb, wbkT_r)
            nc.sync.dma_start(wbv_sb, wbvT_r)
            nc.sync.dma_start(ropeC_sb, ropeC.ap())
            nc.sync.dma_start(ropeS_sb, ropeS.ap())
            nc.sync.dma_start(wo_sb, woT_r)
            nc.sync.dma_start(mask_sb, masks.ap())

            def p1_block(rb, xt, qn3, qp3, kn3, kp3, v3):
                """Projections for row-block rb, written straight into the
                per-batch SBUF tiles attention reads."""
                sl = (rb % 4) * RB     # position within batch

                # ---- q_nope per head + joint q_pe tile ----
                for ft in range(3):
                    ps_q = psA.tile([128, RB], F32, tag="proj", name="ps_q")
                    for dc in range(16):
                        nc.tensor.matmul(ps_q, wq_sb[dc][:, ft * 128:(ft + 1) * 128],
                                         xt[dc], start=(dc == 0), stop=(dc == 15))
                    if ft < 2:
                        nc.scalar.copy(qn3[ft][:, sl:sl + RB], ps_q)
                    else:
                        qpe_sb = sp.tile([128, RB], BF16, tag="pe", name="qpe_sb")
                        nc.scalar.copy(qpe_sb, ps_q)
                        ps_qs = psX.tile([128, RB], F32, tag="aux", name="ps_qs")
                        nc.tensor.matmul(ps_qs, perm_sb, qpe_sb, start=True, stop=True)
                        t1 = sp.tile([128, RB], F32, tag="ropt1", bufs=1, name="t1")
                        t2 = sp.tile([128, RB], F32, tag="ropt2", bufs=1, name="t2")
                        nc.vector.tensor_tensor(t1, qpe_sb,
                                                ropeC_sb[:, sl:sl + RB], MULT)
                        nc.vector.tensor_tensor(t2, ps_qs,
                                                ropeS_sb[:, sl:sl + RB], MULT)
                        nc.vector.tensor_tensor(qp3[:, sl:sl + RB], t1, t2, ADD)

                # ---- kv_c: 4 chunks; sum-of-squares via DVE adds, then a
                # single ones-matmul for the partition reduce (no PSUM hold) --
                kvu = []
                ssacc = sp.tile([128, RB], F32R, tag="ssacc", bufs=1, name="ssacc")
                for ft in range(4):
                    ps_kv = psA.tile([128, RB], F32, tag="proj", name="ps_kv")
                    for dc in range(16):
                        nc.tensor.matmul(ps_kv, wkv_sb[dc][:, ft * 128:(ft + 1) * 128],
                                         xt[dc], start=(dc == 0), stop=(dc == 15))
                    ku = sp.tile([128, RB], BF16, tag=f"kvu{ft}", bufs=1, name=f"ku{ft}")
                    nc.scalar.copy(ku, ps_kv)
                    kvu.append(ku)
                    if ft == 0:
                        nc.vector.tensor_tensor(ssacc, ps_kv, ku, MULT)
                    else:
                        sq = sp.tile([128, RB], F32, tag="sq", bufs=1, name="sq")
                        nc.vector.tensor_tensor(sq, ps_kv, ku, MULT)
                        nc.vector.tensor_tensor(ssacc, ssacc.bitcast(F32), sq, ADD)
                ps_ms = psX.tile([128, RB], F32, tag="aux", name="ps_ms")
                nc.tensor.matmul(ps_ms, ones_sb, ssacc, start=True, stop=True)

                # ---- k_pe: weight columns carry a built-in duplicate, so the
                # projection emits both partition halves at the same matmul
                # cost and no SBUF->SBUF dup DMA sits on the attention path --
                ps_kp = psA.tile([128, RB], F32, tag="proj", name="ps_kp")
                for dc in range(16):
                    nc.tensor.matmul(ps_kp, wkv_sb[dc][:, 512:640],
                                     xt[dc], start=(dc == 0), stop=(dc == 15))
                kpe_sb = sp.tile([128, RB], BF16, tag="kpe", name="kpe_sb")
                nc.scalar.copy(kpe_sb, ps_kp)
                ps_kps = psX.tile([128, RB], F32, tag="aux", name="ps_kps")
                nc.tensor.matmul(ps_kps, perm_sb, kpe_sb, start=True, stop=True)
                k1 = sp.tile([128, RB], F32, tag="kropt1", bufs=1, name="k1")
                nc.vector.tensor_tensor(k1, kpe_sb, ropeC_sb[:, sl:sl + RB], MULT)
                k2 = sp.tile([128, RB], F32, tag="kropt2", bufs=1, name="k2")
                nc.vector.tensor_tensor(k2, ps_kps, ropeS_sb[:, sl:sl + RB], MULT)
                nc.vector.tensor_tensor(kp3[:, sl:sl + RB], k1, k2, ADD)

                # ---- rmsnorm scale: rsqrt via exp(-0.5*ln(y)) (one ACT set) --
                lam = sp.tile([128, RB], F32, tag="lam", bufs=1, name="lam")
                nc.scalar.activation(lam, ps_ms, LN, scale=1.0 / KVR, bias=eps_sb)
                lam2 = sp.tile([128, RB], F32, tag="lam2", bufs=1, name="lam2")
                nc.scalar.activation(lam2, lam, EXP, scale=-0.5)
                kvn = []
                for ft in range(4):
                    kn = sp.tile([128, RB], BF16, tag=f"kvn{ft}", bufs=1, name=f"kvn{ft}")
                    nc.vector.tensor_tensor(kn, kvu[ft], lam2, MULT)
                    kvn.append(kn)

                # ---- fused wkv_b: k_nope^T per head, straight into kn3 ----
                for ht in range(2):
                    ps_k = psA.tile([128, RB], F32, tag="proj", name="ps_k")
                    for kc in range(4):
                        nc.tensor.matmul(ps_k, wbk_sb[:, kc, ht * 128:(ht + 1) * 128],
                                         kvn[kc], start=(kc == 0), stop=(kc == 3))
                    nc.vector.tensor_copy(kn3[ht][:, sl:sl + RB], ps_k)

                # ---- fused wkv_b: v, straight into v3 (row-chunk layout) ----
                for rt in range(4):
                    ps_v = psX.tile([128, RB], F32, tag="aux", name="ps_v")
                    for kc in range(4):
                        nc.tensor.matmul(ps_v[:, :256], kvn[kc][:, rt * 128:(rt + 1) * 128],
                                         wbv_sb[:, kc], start=(kc == 0), stop=(kc == 3))
                    ro = sl // 128 + rt
                    for ht in range(2):
                        nc.vector.tensor_copy(v3[ht][:, ro, :],
                                              ps_v[:, ht * 128:(ht + 1) * 128])

            def p3_chunk(b, qb, qn3, qp3, kn3, kp3, v3):
                """Attention + output projection for q-block qb of batch b.
                Both heads advance together per k-tile: their K=64 rope-score
                matmuls touch disjoint PE row groups and issue back-to-back,
                and two independent exp/PV chains keep the pipeline full."""
                o2 = op3.tile([128, 2, QB], BF16, tag="o2", name="o2")
                nkt = 4 * qb + 4
                sl = qb * QB
                ps_o = [pso.tile([128, QB], F32, tag="o", name=f"ps_o{h}")
                        for h in range(HPC)]
                acc = [[sp3.tile([128, QB], F32R, tag=f"acc{h}{j}", bufs=1,
                                 name=f"acc{h}{j}") for j in range(2)]
                       for h in range(HPC)]
                for kt in range(nkt):
                    m = kt - 4 * qb
                    lo = max(0, m) * 128
                    ps_s = [pss.tile([128, QB], F32, tag="s", name=f"ps_s{h}")
                            for h in range(HPC)]
                    for h in range(HPC):
                        nc.tensor.matmul(ps_s[h][:, lo:],
                                         kn3[h][:, kt * 128:(kt + 1) * 128],
                                         qn3[h][:, sl + lo:sl + QB],
                                         start=True, stop=False)
                    for h in range(HPC):
                        nc.tensor.matmul(ps_s[h][:, lo:],
                                         kp3[h * 64:(h + 1) * 64, kt * 128:(kt + 1) * 128],
                                         qp3[h * 64:(h + 1) * 64, sl + lo:sl + QB],
                                         start=False, stop=True)
                    for h in range(HPC):
                        ex = ep3.tile([128, QB], BF16, tag="ex", name="ex")
                        if lo > 0 and kt < 2:
                            # only the kt<2 full-tile accumulator copy reads
                            # the dead region; everything else is sliced
                            nc.vector.memset(ex[:, :lo], 0.0)
                        nc.scalar.activation(ex[:, lo:], ps_s[h][:, lo:], EXP)
                        if m >= 0:
                            nc.vector.tensor_tensor(
                                ex[:, lo:], ex[:, lo:],
                                mask_sb[:, m * QB + lo:(m + 1) * QB], MULT)
                        # dead columns contribute zero: slice them out of the
                        # PV matmul and the rowsum accumulation (PSUM
                        # has_written keeps the untouched columns' partials)
                        nc.tensor.matmul(ps_o[h][:, lo:], v3[h][:, kt, :],
                                         ex[:, lo:],
                                         start=(kt == 0), stop=(kt == nkt - 1))
                        a = acc[h][kt % 2]
                        if kt < 2:
                            nc.vector.tensor_copy(a, ex)
                        else:
                            nc.vector.tensor_tensor(a[:, lo:],
                                                    a.bitcast(F32)[:, lo:],
                                                    ex[:, lo:], ADD)
                for h in range(HPC):
                    nc.vector.tensor_tensor(acc[h][0], acc[h][0].bitcast(F32),
                                            acc[h][1].bitcast(F32), ADD)
                    ps_r = pss.tile([128, QB], F32, tag="s", name="ps_r")
                    nc.tensor.matmul(ps_r, ones_sb, acc[h][0], start=True, stop=True)
                    rec = sp3.tile([128, QB], F32, tag="rec", name="rec")
                    nc.vector.reciprocal(rec, ps_r)
                    nc.vector.tensor_tensor(o2[:, h], ps_o[h], rec, MULT)

                # ---- output projection for this (b, qb) ----
                for dt_ in range(16):
                    ps_p = pss.tile([128, QB], F32, tag="s", name="ps_p")
                    for hc in range(2):
                        nc.tensor.matmul(ps_p, wo_sb[:, hc, dt_ * 128:(dt_ + 1) * 128],
                                         o2[:, hc], start=(hc == 0), stop=(hc == 1))
                    po = sp4.tile([128, QB], BF16, tag="po", name="po")
                    nc.scalar.copy(po, ps_p)
                    nc.sync.dma_start(
                        pout.ap()[dt_ * 128:(dt_ + 1) * 128,
                                  b * S + qb * QB:b * S + (qb + 1) * QB], po)

            xt_cur = [xt0]
            for b in range(B):
                # persistent per-batch tiles in exactly the attention layout
                qn3 = [kvp.tile([128, S], BF16, tag=f"qn3h{h}", name=f"qn3h{h}")
                       for h in range(HPC)]
                qp3 = kvp.tile([128, S], BF16, tag="qp3", name="qp3")
                kn3 = [kvp.tile([128, S], BF16, tag=f"kn3h{h}", name=f"kn3h{h}")
                       for h in range(HPC)]
                kp3 = kvp.tile([128, S], BF16, tag="kp3", name="kp3")
                v3 = [kvp.tile([128, 16, VD], BF16, tag=f"v3h{h}", name=f"v3h{h}")
                      for h in range(HPC)]
                for qb in range(NQB):
                    rb = 4 * b + qb
                    xt = xt_cur[0]
                    p1_block(rb, xt, qn3, qp3, kn3, kp3, v3)
                    if rb + 1 < NRB:
                        nxt = [xp.tile([128, RB], BF16, tag=f"xt{dc}", name=f"xt{dc}")
                               for dc in range(16)]
                        for dc in range(16):
                            nc.sync.dma_start(nxt[dc],
                                              xT_r[:, dc, (rb + 1) * RB:(rb + 2) * RB])
                        xt_cur[0] = nxt
                    p3_chunk(b, qb, qn3, qp3, kn3, kp3, v3)

    nc.compile()
    return nc


def _prep_inputs(x, wq, wkv_a, kv_norm_w, wkv_b, wo, freqs_cos, freqs_sin):
    import ml_dtypes
    bf16 = ml_dtypes.bfloat16
    x = np.asarray(x, np.float32)
    wq = np.asarray(wq, np.float32)
    wkv_a = np.asarray(wkv_a, np.float32)
    kv_norm_w = np.asarray(kv_norm_w, np.float32)
    wkv_b = np.asarray(wkv_b, np.float32)
    wo = np.asarray(wo, np.float32)
    cos = np.asarray(freqs_cos, np.float32)   # [S, 32]
    sin = np.asarray(freqs_sin, np.float32)

    xT = np.ascontiguousarray(x.reshape(R, D).T).astype(bf16)

    C64 = np.repeat(cos.T, 2, axis=0)         # [64, S]
    S64 = np.repeat(sin.T, 2, axis=0)
    S64 = S64.copy()
    S64[0::2] *= -1.0                         # even rows: -sin; odd: +sin
    ropeC = np.ascontiguousarray(np.vstack([C64, C64])).astype(bf16)   # [128,S]
    ropeS = np.ascontiguousarray(np.vstack([S64, S64])).astype(bf16)

    perm = np.zeros((128, 128), np.float32)
    idx = np.arange(128)
    perm[idx ^ 1, idx] = 1.0                  # out[m] = in[m^1]
    ones = np.ones((128, 128), np.float32)

    # causal mask tiles: masks[p, m*QB + c] = 1.0 iff c >= m*128 + p
    masks = np.zeros((128, 4 * QB), np.float32)
    for m in range(4):
        cgrid = np.arange(QB)[None, :]
        pgrid = np.arange(128)[:, None]
        masks[:, m * QB:(m + 1) * QB] = (cgrid >= m * 128 + pgrid).astype(np.float32)

    wq_h = wq.reshape(H, QK, D)
    wb_h = (wkv_b * kv_norm_w[None, :]).reshape(H, NOPE + VD, KVR)

    in_maps = []
    for c in range(NCORES):
        h0, h1 = 2 * c, 2 * c + 1
        wq_prep = np.concatenate([
            wq_h[h0, :NOPE], wq_h[h1, :NOPE],
            wq_h[h0, NOPE:], wq_h[h1, NOPE:],
        ], axis=0) * SCALE                               # [384, D]
        wbk = np.concatenate([wb_h[h0, :NOPE], wb_h[h1, :NOPE]], axis=0)  # [256,512]
        wbv = np.concatenate([wb_h[h0, NOPE:], wb_h[h1, NOPE:]], axis=0)
        wo_c = np.concatenate([wo[:, h0 * VD:(h0 + 1) * VD],
                               wo[:, h1 * VD:(h1 + 1) * VD]], axis=1)     # [D, 256]
        in_maps.append({
            "xT": xT,
            "wqT": np.ascontiguousarray(wq_prep.T).astype(bf16),
            "wkvT": np.ascontiguousarray(
                np.concatenate([wkv_a, wkv_a[KVR:]], axis=0).T).astype(bf16),
            "wbkT": np.ascontiguousarray(wbk.T).astype(bf16),
            "wbvT": np.ascontiguousarray(wbv.T).astype(bf16),
            "woT": np.ascontiguousarray(wo_c.T).astype(bf16),
            "ropeC": ropeC,
            "ropeS": ropeS,
            "perm": perm.astype(bf16),
            "onesw": ones,
            "masks": masks.astype(bf16),
        })
    return in_maps


def _get_nc():
    if "nc" not in _cache:
        _cache["nc"] = _build_nc()
    return _cache["nc"]


def kernel(**inputs):
    from concourse.bass_utils import run_bass_kernel_spmd
    nc = _get_nc()
    in_maps = _prep_inputs(**inputs)
    res = run_bass_kernel_spmd(nc, in_maps, core_ids=list(range(NCORES)))
    acc = res.results[0]["pout"].astype(np.float64)
    for c in range(1, NCORES):
        acc += res.results[c]["pout"].astype(np.float64)
    return np.ascontiguousarray(acc.T).astype(np.float32).reshape(B, S, D)
